# revision 3
# baseline (speedup 1.0000x reference)
"""GQA attention (RoPE + ALiBi + causal) Bass kernel for Trainium2, 8 NeuronCores.

Sharding: core (b, g) = batch b in {0,1} x kv-group g in {0..3}; each core computes
its 4 query heads' attention for its batch and a partial output projection
(row-parallel wo); host sums the 4 group partials per batch.

Device dataflow (all matmuls in float32r = TF32-rate, fp32 accumulate), fully
pipelined per 512-query window:
  window w: K/V/Q projections (two passes over x, 3 PSUM accumulators each),
  per-window RoPE (DVE) and V transpose (PE), then per head: scoresT = K_u^T Q
  (PSUM), P' = exp(scale*scores + bias_p) in 128-q chunks (ACT per-partition
  bias; the -slope*q half of ALiBi cancels in softmax, leaving a per-kv bias;
  per-chunk recentering keeps exp in fp32 range; diagonal chunks get a causal
  0/1 mask multiply), then outT += V_u^T P' and den += ones^T P'.
  attn = outT * recip(den). Output-projection matmuls of window w-1 are emitted
  as filler inside window w's attention loop so the in-order PE queue stays
  busy while ACT computes exponentials.
"""
import math
from contextlib import ExitStack

import numpy as np

import concourse.bass as bass
import concourse.bacc as bacc
import concourse.tile as tile
from concourse import mybir
from concourse.bass_utils import run_bass_kernel_spmd

F32 = mybir.dt.float32
F32R = mybir.dt.float32r

B, S, D = 2, 2048, 2048
H, KV, HD, REP = 16, 4, 128, 4
NH = 4                     # heads per core
NW = S // 512              # q-windows
ND = D // 128              # d_in tiles
NU = S // 128              # kv tiles
SCALE = 1.0 / math.sqrt(HD)


def build():
    nc = bacc.Bacc(None)
    xT_d = nc.dram_tensor("xT", [D, S], F32R, kind="ExternalInput")
    wq_d = nc.dram_tensor("wqT", [D, NH * HD], F32R, kind="ExternalInput")
    wk_d = nc.dram_tensor("wkT", [D, HD], F32R, kind="ExternalInput")
    wv_d = nc.dram_tensor("wvT", [D, HD], F32R, kind="ExternalInput")
    wo_d = nc.dram_tensor("woT", [NH * HD, D], F32R, kind="ExternalInput")
    cosF_d = nc.dram_tensor("cosF", [128, S], F32R, kind="ExternalInput")
    sinF_d = nc.dram_tensor("sinF", [128, S], F32R, kind="ExternalInput")
    biasb_d = nc.dram_tensor("biasb", [128, NH * 16], F32, kind="ExternalInput")
    cmask_d = nc.dram_tensor("cmask", [128, 128], F32R, kind="ExternalInput")
    ident_d = nc.dram_tensor("ident", [128, 128], F32, kind="ExternalInput")
    ones_d = nc.dram_tensor("ones", [128, 128], F32R, kind="ExternalInput")
    part_d = nc.dram_tensor("part", [S, D], F32, kind="ExternalOutput")

    PSUM = bass.MemorySpace.PSUM

    with tile.TileContext(nc) as tc:
        with ExitStack() as ctx:
            consts = ctx.enter_context(tc.tile_pool(name="consts", bufs=1))
            persist = ctx.enter_context(tc.tile_pool(name="persist", bufs=1))
            wpool = ctx.enter_context(tc.tile_pool(name="wpool", bufs=1))
            xpool = ctx.enter_context(tc.tile_pool(name="xpool", bufs=6))
            qtp = ctx.enter_context(tc.tile_pool(name="qtp", bufs=6))
            atp = ctx.enter_context(tc.tile_pool(name="atp", bufs=8))
            vtp = ctx.enter_context(tc.tile_pool(name="vtp", bufs=2))
            rp = ctx.enter_context(tc.tile_pool(name="rp", bufs=2))
            Pp = ctx.enter_context(tc.tile_pool(name="Pp", bufs=5))
            ep = ctx.enter_context(tc.tile_pool(name="ep", bufs=3))
            ostg = ctx.enter_context(tc.tile_pool(name="ostg", bufs=3))
            pj = ctx.enter_context(tc.tile_pool(name="pj", bufs=3, space=PSUM))
            spp = ctx.enter_context(tc.tile_pool(name="spp", bufs=2, space=PSUM))
            dpp = ctx.enter_context(tc.tile_pool(name="dpp", bufs=1, space=PSUM))
            opp = ctx.enter_context(tc.tile_pool(name="opp", bufs=1, space=PSUM))
            ojp = ctx.enter_context(tc.tile_pool(name="ojp", bufs=1, space=PSUM))

            # weights, split per d-tile so the first matmuls start early
            wq_sb = wpool.tile([128, ND, NH * HD], F32R, tag="wq")
            wk_sb = wpool.tile([128, ND, HD], F32R, tag="wk")
            wv_sb = wpool.tile([128, ND, HD], F32R, tag="wv")
            wqr = wq_d.rearrange("(t p) o -> p t o", p=128)
            wkr = wk_d.rearrange("(t p) o -> p t o", p=128)
            wvr = wv_d.rearrange("(t p) o -> p t o", p=128)
            for d in range(ND):
                nc.sync.dma_start(wk_sb[:, d, :], wkr[:, d, :])
                nc.sync.dma_start(wv_sb[:, d, :], wvr[:, d, :])
                nc.sync.dma_start(wq_sb[:, d, :], wqr[:, d, :])

            cosF = consts.tile([128, S], F32R, tag="cosF")
            sinF = consts.tile([128, S], F32R, tag="sinF")
            biasb = consts.tile([128, NH * 16], F32, tag="biasb")
            cmask = consts.tile([128, 128], F32R, tag="cmask")
            ident = consts.tile([128, 128], F32, tag="ident")
            ones = consts.tile([128, 128], F32R, tag="ones")
            nc.sync.dma_start(cosF[:], cosF_d[:])
            nc.sync.dma_start(sinF[:], sinF_d[:])
            nc.sync.dma_start(biasb[:], biasb_d[:])
            nc.sync.dma_start(cmask[:], cmask_d[:])
            nc.sync.dma_start(ident[:], ident_d[:])
            nc.sync.dma_start(ones[:], ones_d[:])

            wo_sb = wpool.tile([128, NH, D], F32R, tag="wo")
            nc.sync.dma_start(wo_sb[:], wo_d.rearrange("(h p) o -> p h o", p=128))

            kT = persist.tile([128, S], F32R, tag="kT")
            vnat = persist.tile([128, S], F32R, tag="vnat")

            filler_q = []

            def emit_fillers(n):
                for _ in range(n):
                    if not filler_q:
                        return
                    filler_q.pop(0)()

            def rope(tgt_ap, qsl):
                """tgt_ap: [128, 512] f32r slice view; in-place rotate."""
                qb = rp.tile([128, 512], F32R, tag="qb", name="qb")
                nc.sync.dma_start(qb[0:64, :], tgt_ap[64:128, :])
                nc.sync.dma_start(qb[64:128, :], tgt_ap[0:64, :])
                t1 = rp.tile([128, 512], F32R, tag="t1", name="t1")
                nc.vector.tensor_mul(t1[:], tgt_ap[:], cosF[:, qsl])
                nc.vector.tensor_mul(qb[:], qb[:], sinF[:, qsl])
                nc.vector.tensor_add(tgt_ap[:], t1[:], qb[:])

            attnr = {}
            for w in range(NW):
                qsl = slice(w * 512, (w + 1) * 512)
                U = 4 * (w + 1)

                # ---- projection pass 1: k, v, q0 ----
                pk = pj.tile([128, 512], F32, tag="pj", name=f"pk{w}")
                pv = pj.tile([128, 512], F32, tag="pj", name=f"pv{w}")
                pq0 = pj.tile([128, 512], F32, tag="pj", name=f"pq0_{w}")
                for d in range(ND):
                    xs = xpool.tile([128, 512], F32R, tag="x", name="xs")
                    nc.sync.dma_start(xs[:], xT_d[d * 128:(d + 1) * 128, qsl])
                    st, sp = (d == 0), (d == ND - 1)
                    nc.tensor.matmul(pk[:], wk_sb[:, d, :], xs[:], start=st, stop=sp)
                    nc.tensor.matmul(pv[:], wv_sb[:, d, :], xs[:], start=st, stop=sp)
                    nc.tensor.matmul(pq0[:], wq_sb[:, d, 0:128], xs[:], start=st, stop=sp)
                nc.vector.tensor_copy(kT[:, qsl], pk[:])
                vTw = vtp.tile([128, 512], F32, tag="vT", name=f"vT{w}")
                nc.vector.tensor_copy(vTw[:], pv[:])
                qTr = [qtp.tile([128, 512], F32R, tag="qTr", name=f"qTr{w}_{h}")
                       for h in range(NH)]
                nc.vector.tensor_copy(qTr[0][:], pq0[:])

                # ---- projection pass 2: q1, q2, q3 ----
                pq1 = pj.tile([128, 512], F32, tag="pj", name=f"pq1_{w}")
                pq2 = pj.tile([128, 512], F32, tag="pj", name=f"pq2_{w}")
                pq3 = pj.tile([128, 512], F32, tag="pj", name=f"pq3_{w}")
                pqs = [pq1, pq2, pq3]
                for d in range(ND):
                    xs = xpool.tile([128, 512], F32R, tag="x", name="xs2")
                    nc.sync.dma_start(xs[:], xT_d[d * 128:(d + 1) * 128, qsl])
                    st, sp = (d == 0), (d == ND - 1)
                    for i, pqx in enumerate(pqs):
                        nc.tensor.matmul(pqx[:], wq_sb[:, d, (i + 1) * 128:(i + 2) * 128],
                                         xs[:], start=st, stop=sp)
                for i, pqx in enumerate(pqs):
                    nc.vector.tensor_copy(qTr[i + 1][:], pqx[:])

                # ---- V transpose into vnat ----
                for j in range(4):
                    tp = ojp.tile([128, 128], F32, tag="oj", name=f"tp{w}_{j}")
                    nc.tensor.transpose(tp[:], vTw[:, j * 128:(j + 1) * 128], ident[:])
                    nc.vector.tensor_copy(vnat[:, (4 * w + j) * 128:(4 * w + j + 1) * 128],
                                          tp[:])

                # ---- RoPE ----
                rope(kT[:, qsl], qsl)
                for h in range(NH):
                    rope(qTr[h][:], qsl)

                # ---- attention ----
                for h in range(NH):
                    o_ps = opp.tile([128, 512], F32, tag="o", name=f"o{w}_{h}")
                    d_ps = dpp.tile([128, 512], F32, tag="den", name=f"den{w}_{h}")
                    pend = None
                    for u in range(U):
                        i0 = max(0, u - 4 * w)
                        n0 = 128 * i0
                        s_ps = spp.tile([128, 512], F32, tag="s", name="s")
                        nc.tensor.matmul(
                            s_ps[:, n0:512],
                            kT[:, u * 128:(u + 1) * 128],
                            qTr[h][:, n0:512],
                            start=True, stop=True)
                        Pt = Pp.tile([128, 512], F32R, tag="P", name="P")
                        for i in range(i0, 4):
                            t = 4 * w + i - u
                            csl = slice(i * 128, (i + 1) * 128)
                            nc.scalar.activation(
                                Pt[:, csl], s_ps[:, csl],
                                mybir.ActivationFunctionType.Exp,
                                bias=biasb[:, h * 16 + t:h * 16 + t + 1],
                                scale=SCALE)
                            if t == 0:
                                nc.vector.tensor_mul(Pt[:, csl], Pt[:, csl], cmask[:])
                        emit_fillers(1)
                        if pend is not None:
                            pPt, pn0, pu = pend
                            nc.tensor.matmul(o_ps[:, pn0:512],
                                             vnat[:, pu * 128:(pu + 1) * 128],
                                             pPt[:, pn0:512],
                                             start=(pu == 0), stop=False)
                            nc.tensor.matmul(d_ps[:, pn0:512], ones[:],
                                             pPt[:, pn0:512],
                                             start=(pu == 0), stop=False)
                        pend = (Pt, n0, u)
                    pPt, pn0, pu = pend
                    nc.tensor.matmul(o_ps[:, pn0:512],
                                     vnat[:, pu * 128:(pu + 1) * 128],
                                     pPt[:, pn0:512], start=(pu == 0), stop=True)
                    nc.tensor.matmul(d_ps[:, pn0:512], ones[:],
                                     pPt[:, pn0:512], start=(pu == 0), stop=True)
                    rec = ep.tile([128, 512], F32, tag="rec", name="rec")
                    nc.vector.reciprocal_approx_fast(rec[:], d_ps[:])
                    at = atp.tile([128, 512], F32R, tag="attnr", name=f"at{w}_{h}")
                    nc.vector.tensor_mul(at[:], o_ps[:], rec[:])
                    attnr[(h, w)] = at

                # ---- enqueue output projection for this window as filler ----
                def make_unit(w_, mq_, dwin_):
                    def unit():
                        po = ojp.tile([128, 512], F32, tag="oj",
                                      name=f"po{w_}_{mq_}_{dwin_}")
                        for h_ in range(NH):
                            nc.tensor.matmul(
                                po[:],
                                attnr[(h_, w_)][:, mq_ * 128:(mq_ + 1) * 128],
                                wo_sb[:, h_, dwin_ * 512:(dwin_ + 1) * 512],
                                start=(h_ == 0), stop=(h_ == NH - 1))
                        so = ostg.tile([128, 512], F32, tag="so", name="so")
                        nc.vector.tensor_copy(so[:], po[:])
                        m_ = 4 * w_ + mq_
                        nc.sync.dma_start(
                            part_d[m_ * 128:(m_ + 1) * 128,
                                   dwin_ * 512:(dwin_ + 1) * 512], so[:])
                    return unit

                for mq in range(4):
                    for dwin in range(4):
                        filler_q.append(make_unit(w, mq, dwin))

            emit_fillers(len(filler_q))
    nc.finalize()
    return nc


_NC_CACHE = {}


def _get_nc():
    if "nc" not in _NC_CACHE:
        _NC_CACHE["nc"] = build()
    return _NC_CACHE["nc"]


def _host_prep(x, alibi_bias, wq, wk, wv, wo):
    """Build per-core input maps (shard + transpose + rope tables + bias tables)."""
    x = np.asarray(x, np.float32)
    alibi_bias = np.asarray(alibi_bias, np.float32)
    wq = np.asarray(wq, np.float32)
    wk = np.asarray(wk, np.float32)
    wv = np.asarray(wv, np.float32)
    wo = np.asarray(wo, np.float32)

    slopes = alibi_bias[0, :, 0, 1].copy()        # [H]; alibi[0,h,0,1] = slope_h

    inv_freq = 1.0 / (10000.0 ** (np.arange(0, HD, 2, dtype=np.float32) / HD))
    t = np.arange(S, dtype=np.float32)
    freqs = np.outer(t, inv_freq)                 # [S, 64]
    cos = np.cos(freqs).astype(np.float32).T      # [64, S]
    sin = np.sin(freqs).astype(np.float32).T
    cosF = np.ascontiguousarray(np.concatenate([cos, cos], 0))     # [128, S]
    sinF = np.ascontiguousarray(np.concatenate([-sin, sin], 0))

    perm = np.concatenate([np.arange(0, HD, 2), np.arange(1, HD, 2)])
    p_ar = np.arange(128, dtype=np.float32)
    cmask = (p_ar[:, None] <= p_ar[None, :]).astype(np.float32)
    ident = np.eye(128, dtype=np.float32)
    ones = np.ones((128, 128), np.float32)

    xTs = [np.ascontiguousarray(x[b].T) for b in range(B)]
    in_maps = []
    for core in range(8):
        b, g = divmod(core, KV)
        wq_g = wq[4 * g * HD:(4 * g + 4) * HD].reshape(NH, HD, D)[:, perm, :]
        wqT = np.ascontiguousarray(wq_g.reshape(NH * HD, D).T)
        wkT = np.ascontiguousarray(wk[g * HD:(g + 1) * HD][perm].T)
        wvT = np.ascontiguousarray(wv[g * HD:(g + 1) * HD].T)
        woT = np.ascontiguousarray(wo[:, 4 * g * HD:(4 * g + 4) * HD].T)
        biasb = np.zeros((128, NH * 16), np.float32)
        for h in range(NH):
            sl = slopes[4 * g + h]
            for tt in range(16):
                biasb[:, h * 16 + tt] = sl * (p_ar - 96.0 - 128.0 * tt)
        in_maps.append({
            "xT": xTs[b], "wqT": wqT, "wkT": wkT, "wvT": wvT, "woT": woT,
            "cosF": cosF, "sinF": sinF, "biasb": biasb, "cmask": cmask,
            "ident": ident, "ones": ones,
        })
    return in_maps


def kernel(x, mask, alibi_bias, wq, wk, wv, wo, _trace=False, _trace_kwargs=None):
    nc = _get_nc()
    in_maps = _host_prep(x, alibi_bias, wq, wk, wv, wo)
    res = run_bass_kernel_spmd(nc, in_maps, list(range(8)), trace=_trace,
                               **(_trace_kwargs or {}))
    parts = [res.results[c]["part"] for c in range(8)]
    out = np.stack([
        parts[0] + parts[1] + parts[2] + parts[3],
        parts[4] + parts[5] + parts[6] + parts[7],
    ]).astype(np.float32)
    if _trace:
        return out, res
    return out
